# revision 8
# baseline (speedup 1.0000x reference)
"""Trainium2 Bass kernel for nn_Decoder (ragged_sequence).

Reference math (B=8192, HID=256, MAX_N=64, DIM=128):
    h        = relu(LN(z @ sW1 + sb1) * sg + sbe)          # [B, 128]
    n_logits = h @ sW2 + sb2                               # [B, 1]
    n        = clip(round(n_logits), 0, 63).int32          # [B]
    keys     = tanh(LN(kW1 + kb1) * kg + kbe) @ kW2 + kb2  # [64, HID]  (pos=I)
    x[b,j]   = (tanh((z[b]*keys[j]) @ dW1 + db1) @ dW2 + db2) * (j < n[b])

The ragged structure means almost every (b, j) slot is masked to zero
(n is ~0-2 for this distribution).  The kernel plans a compile-time slot
cap J_CAP = max(n)+1 from a host fp64 replica of the size-MLP, computes
only those slots on device (per-row mask still applied on device from
the device-computed n), and the host assembles the full dense
[B, 64, 128] output with np.zeros for the never-valid slots.

Layout: batch lives on the matmul *free* dimension through the whole
chain (zT [HID, batch] built once via PE transposes), so dW1/dW2/sW1/kW2
are used in their native [K, M] layout as stationary operands, biases
fall on partitions (ACT bias), and a single PE transpose per output
block returns batch to partitions where the per-row mask is a
tensor_scalar multiply during PSUM eviction.

Sharding: pure data parallel over batch across 8 cores; weights
replicated.
"""

from contextlib import ExitStack

import numpy as np

import concourse.bacc as bacc
import concourse.bass as bass
import concourse.tile as tile
from concourse import mybir
from concourse.alu_op_type import AluOpType
from concourse.bass_utils import run_bass_kernel_spmd
from concourse.masks import make_identity

F32 = mybir.dt.float32
I32 = mybir.dt.int32
AF = mybir.ActivationFunctionType

B, HID, MAX_N, DIM = 8192, 256, 64, 128
KM, DM, SM = 160, 192, 128
N_CORES = 8
BC = B // N_CORES          # 1024 rows per core
ST = 512                   # supertile: batch rows per matmul free dim
N_ST = BC // ST            # 2
N_BT = BC // 128           # 8 batch tiles of 128 rows
LN_EPS = 1e-5
MAGIC = 12582912.0         # 1.5 * 2^23: fp32 round-to-nearest-even trick

_CACHE: dict[int, object] = {}


def _build(j_cap: int):
    """Build + compile the per-core Bass program for a slot cap of j_cap."""
    nc = bacc.Bacc("TRN2", target_bir_lowering=False, debug=False,
                   num_devices=N_CORES)

    # ---- DRAM I/O (per-core shapes; names match setup_inputs keys) ----
    z_d = nc.dram_tensor("z", [BC, HID], F32, kind="ExternalInput").ap()
    kW1_d = nc.dram_tensor("kW1", [MAX_N, KM], F32, kind="ExternalInput").ap()
    kb1_d = nc.dram_tensor("kb1", [KM], F32, kind="ExternalInput").ap()
    kg_d = nc.dram_tensor("kg", [KM], F32, kind="ExternalInput").ap()
    kbe_d = nc.dram_tensor("kbe", [KM], F32, kind="ExternalInput").ap()
    kW2_d = nc.dram_tensor("kW2", [KM, HID], F32, kind="ExternalInput").ap()
    kb2_d = nc.dram_tensor("kb2", [HID], F32, kind="ExternalInput").ap()
    dW1_d = nc.dram_tensor("dW1", [HID, DM], F32, kind="ExternalInput").ap()
    db1_d = nc.dram_tensor("db1", [DM], F32, kind="ExternalInput").ap()
    dW2_d = nc.dram_tensor("dW2", [DM, DIM], F32, kind="ExternalInput").ap()
    db2_d = nc.dram_tensor("db2", [DIM], F32, kind="ExternalInput").ap()
    sW1_d = nc.dram_tensor("sW1", [HID, SM], F32, kind="ExternalInput").ap()
    sb1_d = nc.dram_tensor("sb1", [SM], F32, kind="ExternalInput").ap()
    sg_d = nc.dram_tensor("sg", [SM], F32, kind="ExternalInput").ap()
    sbe_d = nc.dram_tensor("sbe", [SM], F32, kind="ExternalInput").ap()
    sW2_d = nc.dram_tensor("sW2", [SM], F32, kind="ExternalInput").ap()
    sb2_d = nc.dram_tensor("sb2", [1], F32, kind="ExternalInput").ap()

    x_d = nc.dram_tensor("xout", [BC, j_cap * DIM], F32,
                         kind="ExternalOutput").ap()
    nl_d = nc.dram_tensor("nlout", [BC, 1], F32, kind="ExternalOutput").ap()
    n_d = nc.dram_tensor("nout", [BC, 1], I32, kind="ExternalOutput").ap()

    def bc_ap(ap1d, p):
        # [F] DRAM view -> [p, F] partition-broadcast AP
        return bass.AP(tensor=ap1d.tensor, offset=ap1d.offset,
                       ap=[[0, p]] + [list(e) for e in ap1d.ap])

    def col_ap(ap1d):
        # [F] DRAM view -> [F, 1] column AP (partition-major)
        return bass.AP(tensor=ap1d.tensor, offset=ap1d.offset,
                       ap=[list(ap1d.ap[0]), [1, 1]])

    with tile.TileContext(nc) as tc, ExitStack() as es:
        singles = es.enter_context(tc.tile_pool(name="singles", bufs=1))
        work = es.enter_context(tc.tile_pool(name="work", bufs=3))
        zwork = es.enter_context(tc.tile_pool(name="zwork", bufs=3))
        jwork = es.enter_context(tc.tile_pool(name="jwork", bufs=3))
        outp = es.enter_context(tc.tile_pool(name="outp", bufs=N_BT))
        ps_small = es.enter_context(
            tc.tile_pool(name="ps_small", bufs=2, space="PSUM"))
        ps_y0 = es.enter_context(tc.tile_pool(name="ps_y0", bufs=2, space="PSUM"))
        ps_y1 = es.enter_context(tc.tile_pool(name="ps_y1", bufs=2, space="PSUM"))
        ps_x = es.enter_context(tc.tile_pool(name="ps_x", bufs=2, space="PSUM"))

        # ---------------- constants ----------------
        ident = singles.tile([128, 128], F32, tag="ident")
        make_identity(nc, ident)

        dW1_k0 = singles.tile([128, DM], F32, tag="dW1k0")
        dW1_k1 = singles.tile([128, DM], F32, tag="dW1k1")
        nc.sync.dma_start(out=dW1_k0, in_=dW1_d[0:128, :])
        nc.sync.dma_start(out=dW1_k1, in_=dW1_d[128:256, :])
        dW2_k0 = singles.tile([128, DIM], F32, tag="dW2k0")
        dW2_k1 = singles.tile([64, DIM], F32, tag="dW2k1")
        nc.sync.dma_start(out=dW2_k0, in_=dW2_d[0:128, :])
        nc.sync.dma_start(out=dW2_k1, in_=dW2_d[128:192, :])
        sW1_k0 = singles.tile([128, SM], F32, tag="sW1k0")
        sW1_k1 = singles.tile([128, SM], F32, tag="sW1k1")
        nc.sync.dma_start(out=sW1_k0, in_=sW1_d[0:128, :])
        nc.sync.dma_start(out=sW1_k1, in_=sW1_d[128:256, :])
        kW1_t = singles.tile([MAX_N, KM], F32, tag="kW1")
        nc.sync.dma_start(out=kW1_t, in_=kW1_d)
        kW2_a = singles.tile([128, HID], F32, tag="kW2a")
        kW2_b = singles.tile([32, HID], F32, tag="kW2b")
        nc.sync.dma_start(out=kW2_a, in_=kW2_d[0:128, :])
        nc.sync.dma_start(out=kW2_b, in_=kW2_d[128:160, :])

        db1_c0 = singles.tile([128, 1], F32, tag="db1c0")
        db1_c1 = singles.tile([64, 1], F32, tag="db1c1")
        nc.sync.dma_start(out=db1_c0, in_=col_ap(db1_d[0:128]))
        nc.sync.dma_start(out=db1_c1, in_=col_ap(db1_d[128:192]))
        db2_c = singles.tile([128, 1], F32, tag="db2c")
        nc.sync.dma_start(out=db2_c, in_=col_ap(db2_d))
        kb2_c0 = singles.tile([128, 1], F32, tag="kb2c0")
        kb2_c1 = singles.tile([128, 1], F32, tag="kb2c1")
        nc.sync.dma_start(out=kb2_c0, in_=col_ap(kb2_d[0:128]))
        nc.sync.dma_start(out=kb2_c1, in_=col_ap(kb2_d[128:256]))

        sb1_b = singles.tile([128, SM], F32, tag="sb1b")
        sg_b = singles.tile([128, SM], F32, tag="sgb")
        sbe_b = singles.tile([128, SM], F32, tag="sbeb")
        sW2_b = singles.tile([128, SM], F32, tag="sW2b")
        sb2_c = singles.tile([128, 1], F32, tag="sb2c")
        nc.sync.dma_start(out=sb1_b, in_=bc_ap(sb1_d, 128))
        nc.sync.dma_start(out=sg_b, in_=bc_ap(sg_d, 128))
        nc.sync.dma_start(out=sbe_b, in_=bc_ap(sbe_d, 128))
        nc.sync.dma_start(out=sW2_b, in_=bc_ap(sW2_d, 128))
        nc.sync.dma_start(out=sb2_c, in_=bc_ap(sb2_d, 128))
        kb1_b = singles.tile([MAX_N, KM], F32, tag="kb1b")
        kg_b = singles.tile([MAX_N, KM], F32, tag="kgb")
        kbe_b = singles.tile([MAX_N, KM], F32, tag="kbeb")
        nc.sync.dma_start(out=kb1_b, in_=bc_ap(kb1_d, MAX_N))
        nc.sync.dma_start(out=kg_b, in_=bc_ap(kg_d, MAX_N))
        nc.sync.dma_start(out=kbe_b, in_=bc_ap(kbe_d, MAX_N))

        eps128 = singles.tile([128, 1], F32, tag="eps128")
        nc.vector.memset(eps128, LN_EPS)

        iota_i = singles.tile([128, j_cap], I32, tag="iotai")
        nc.gpsimd.iota(out=iota_i, pattern=[[1, j_cap]], base=0,
                       channel_multiplier=0)
        iota_f = singles.tile([128, j_cap], F32, tag="iotaf")
        nc.vector.tensor_copy(out=iota_f, in_=iota_i)

        # persistent per-core state
        zT0 = singles.tile([128, BC], F32, tag="zT0")   # z.T rows 0:128
        zT1 = singles.tile([128, BC], F32, tag="zT1")   # z.T rows 128:256
        keysT0 = singles.tile([128, MAX_N], F32, tag="keysT0")
        keysT1 = singles.tile([128, MAX_N], F32, tag="keysT1")
        mask_all = singles.tile([128, N_BT, j_cap], F32, tag="maskall")

        # ---------------- keys net (once) ----------------
        kt = work.tile([MAX_N, KM], F32, tag="ktmp")
        nc.vector.tensor_tensor(out=kt, in0=kW1_t, in1=kb1_b, op=AluOpType.add)
        kstats = work.tile([MAX_N, 6], F32, tag="kstats")
        nc.vector.bn_stats(out=kstats, in_=kt)
        kmv = work.tile([MAX_N, 2], F32, tag="kmv")
        nc.vector.bn_aggr(out=kmv, in_=kstats)
        ksd = work.tile([MAX_N, 1], F32, tag="ksd")
        nc.scalar.activation(out=ksd, in_=kmv[:, 1:2], func=AF.Sqrt,
                             bias=eps128[0:MAX_N, :])
        krstd = work.tile([MAX_N, 1], F32, tag="krstd")
        nc.vector.reciprocal(out=krstd, in_=ksd)
        kt1 = work.tile([MAX_N, KM], F32, tag="kt1")
        nc.vector.scalar_tensor_tensor(out=kt1, in0=kt, scalar=kmv[:, 0:1],
                                       in1=kg_b, op0=AluOpType.subtract,
                                       op1=AluOpType.mult)
        kt2 = work.tile([MAX_N, KM], F32, tag="kt2")
        nc.vector.scalar_tensor_tensor(out=kt2, in0=kt1, scalar=krstd,
                                       in1=kbe_b, op0=AluOpType.mult,
                                       op1=AluOpType.add)
        kth = work.tile([MAX_N, KM], F32, tag="kth")
        nc.scalar.activation(out=kth, in_=kt2, func=AF.Tanh)
        # transpose tanh(LN(...)) -> [KM, 64] in two chunks
        thT_a = work.tile([128, MAX_N], F32, tag="thTa")
        thT_b = work.tile([32, MAX_N], F32, tag="thTb")
        pt = ps_small.tile([128, MAX_N], F32, tag="small")
        nc.tensor.transpose(pt, kth[:, 0:128], ident[0:MAX_N, 0:MAX_N])
        nc.vector.tensor_copy(out=thT_a, in_=pt)
        pt = ps_small.tile([32, MAX_N], F32, tag="small")
        nc.tensor.transpose(pt, kth[:, 128:160], ident[0:MAX_N, 0:MAX_N])
        nc.vector.tensor_copy(out=thT_b, in_=pt)
        # keysT = kW2.T @ thT (+ kb2), hid chunks of 128
        for h, (keysT, kb2c) in enumerate(((keysT0, kb2_c0), (keysT1, kb2_c1))):
            pk = ps_small.tile([128, MAX_N], F32, tag="small")
            nc.tensor.matmul(pk, lhsT=kW2_a[:, h * 128:(h + 1) * 128],
                             rhs=thT_a, start=True, stop=False)
            nc.tensor.matmul(pk, lhsT=kW2_b[:, h * 128:(h + 1) * 128],
                             rhs=thT_b, start=False, stop=True)
            nc.scalar.activation(out=keysT, in_=pk, func=AF.Identity, bias=kb2c)

        # ---------------- zT build + size MLP per batch tile ----------------
        for bt in range(N_BT):
            sl = slice(bt * 128, (bt + 1) * 128)
            zt = zwork.tile([128, HID], F32, tag="zin")
            nc.sync.dma_start(out=zt, in_=z_d[sl, :])
            for h, zT in enumerate((zT0, zT1)):
                pz = ps_small.tile([128, 128], F32, tag="small")
                nc.tensor.transpose(pz, zt[:, h * 128:(h + 1) * 128], ident)
                nc.vector.tensor_copy(out=zT[:, sl], in_=pz)

            # H = z @ sW1 : [128 rows, SM]
            ph = ps_small.tile([128, SM], F32, tag="small")
            nc.tensor.matmul(ph, lhsT=zT0[:, sl], rhs=sW1_k0,
                             start=True, stop=False)
            nc.tensor.matmul(ph, lhsT=zT1[:, sl], rhs=sW1_k1,
                             start=False, stop=True)
            hpre = work.tile([128, SM], F32, tag="hpre")
            nc.vector.tensor_tensor(out=hpre, in0=ph, in1=sb1_b,
                                    op=AluOpType.add)
            sstats = work.tile([128, 6], F32, tag="sstats")
            nc.vector.bn_stats(out=sstats, in_=hpre)
            smv = work.tile([128, 2], F32, tag="smv")
            nc.vector.bn_aggr(out=smv, in_=sstats)
            ssd = work.tile([128, 1], F32, tag="ssd")
            nc.scalar.activation(out=ssd, in_=smv[:, 1:2], func=AF.Sqrt,
                                 bias=eps128)
            srstd = work.tile([128, 1], F32, tag="srstd")
            nc.vector.reciprocal(out=srstd, in_=ssd)
            st1 = work.tile([128, SM], F32, tag="st1")
            nc.vector.scalar_tensor_tensor(out=st1, in0=hpre,
                                           scalar=smv[:, 0:1], in1=sg_b,
                                           op0=AluOpType.subtract,
                                           op1=AluOpType.mult)
            st2 = work.tile([128, SM], F32, tag="st2")
            nc.vector.scalar_tensor_tensor(out=st2, in0=st1, scalar=srstd,
                                           in1=sbe_b, op0=AluOpType.mult,
                                           op1=AluOpType.add)
            hr = work.tile([128, SM], F32, tag="hr")
            nc.scalar.activation(out=hr, in_=st2, func=AF.Relu)
            dot = work.tile([128, SM], F32, tag="dot")
            nc.vector.tensor_tensor(out=dot, in0=hr, in1=sW2_b,
                                    op=AluOpType.mult)
            nl = work.tile([128, 1], F32, tag="nl")
            nc.vector.tensor_reduce(out=nl, in_=dot, axis=mybir.AxisListType.X,
                                    op=AluOpType.add)
            nl2 = work.tile([128, 1], F32, tag="nl2")
            nc.vector.tensor_scalar(out=nl2, in0=nl, scalar1=sb2_c,
                                    scalar2=None, op0=AluOpType.add)
            nc.sync.dma_start(out=nl_d[sl, :], in_=nl2)
            # n = clip(round_half_even(nl2), 0, 63)
            nfa = work.tile([128, 1], F32, tag="nfa")
            nc.vector.tensor_scalar(out=nfa, in0=nl2, scalar1=MAGIC,
                                    scalar2=None, op0=AluOpType.add)
            nf = work.tile([128, 1], F32, tag="nf")
            nc.vector.tensor_scalar(out=nf, in0=nfa, scalar1=-MAGIC,
                                    scalar2=None, op0=AluOpType.add)
            nfc = work.tile([128, 1], F32, tag="nfc")
            nc.vector.tensor_scalar(out=nfc, in0=nf, scalar1=0.0,
                                    scalar2=float(MAX_N - 1),
                                    op0=AluOpType.max, op1=AluOpType.min)
            ni = work.tile([128, 1], I32, tag="ni")
            nc.vector.tensor_copy(out=ni, in_=nfc)
            nc.sync.dma_start(out=n_d[sl, :], in_=ni)
            # mask[p, j] = (j < n[p])
            nc.vector.tensor_scalar(out=mask_all[:, bt, :], in0=iota_f,
                                    scalar1=nfc, scalar2=None,
                                    op0=AluOpType.is_lt)

        # ---------------- decoder: j_cap slots per supertile ----------------
        for st in range(N_ST):
            ssl = slice(st * ST, (st + 1) * ST)
            obig = [outp.tile([128, j_cap * DIM], F32, tag="obig",
                              name=f"obig_{st}_{c}")
                    for c in range(ST // 128)]
            for j in range(j_cap):
                zj0 = jwork.tile([128, ST], F32, tag="zj0")
                zj1 = jwork.tile([128, ST], F32, tag="zj1")
                nc.scalar.activation(out=zj0, in_=zT0[:, ssl], func=AF.Copy,
                                     scale=keysT0[:, j:j + 1])
                nc.scalar.activation(out=zj1, in_=zT1[:, ssl], func=AF.Copy,
                                     scale=keysT1[:, j:j + 1])
                py0 = ps_y0.tile([128, ST], F32, tag="py0")
                nc.tensor.matmul(py0, lhsT=dW1_k0[:, 0:128], rhs=zj0,
                                 start=True, stop=False)
                nc.tensor.matmul(py0, lhsT=dW1_k1[:, 0:128], rhs=zj1,
                                 start=False, stop=True)
                py1 = ps_y1.tile([64, ST], F32, tag="py1")
                nc.tensor.matmul(py1, lhsT=dW1_k0[:, 128:192], rhs=zj0,
                                 start=True, stop=False)
                nc.tensor.matmul(py1, lhsT=dW1_k1[:, 128:192], rhs=zj1,
                                 start=False, stop=True)
                y0 = jwork.tile([128, ST], F32, tag="y0")
                y1 = jwork.tile([64, ST], F32, tag="y1")
                nc.scalar.activation(out=y0, in_=py0, func=AF.Tanh,
                                     bias=db1_c0)
                nc.scalar.activation(out=y1, in_=py1, func=AF.Tanh,
                                     bias=db1_c1)
                px = ps_x.tile([128, ST], F32, tag="px")
                nc.tensor.matmul(px, lhsT=dW2_k0, rhs=y0,
                                 start=True, stop=False)
                nc.tensor.matmul(px, lhsT=dW2_k1, rhs=y1,
                                 start=False, stop=True)
                xt = jwork.tile([128, ST], F32, tag="xt")
                nc.vector.tensor_scalar(out=xt, in0=px, scalar1=db2_c,
                                        scalar2=None, op0=AluOpType.add)
                for c in range(ST // 128):
                    bt = st * (ST // 128) + c
                    pxt = ps_small.tile([128, 128], F32, tag="small")
                    nc.tensor.transpose(pxt, xt[:, c * 128:(c + 1) * 128],
                                        ident)
                    nc.vector.tensor_scalar(
                        out=obig[c][:, j * DIM:(j + 1) * DIM], in0=pxt,
                        scalar1=mask_all[:, bt, j:j + 1], scalar2=None,
                        op0=AluOpType.mult)
            for c in range(ST // 128):
                bt = st * (ST // 128) + c
                nc.sync.dma_start(
                    out=x_d[bt * 128:(bt + 1) * 128, :], in_=obig[c])

    nc.compile()
    return nc


def _host_n_max(z, sW1, sb1, sg, sbe, sW2, sb2):
    """fp64 replica of the size-MLP to plan the compile-time slot cap."""
    h = z.astype(np.float64) @ sW1.astype(np.float64) + sb1.astype(np.float64)
    mu = h.mean(-1, keepdims=True)
    v = ((h - mu) ** 2).mean(-1, keepdims=True)
    h = (h - mu) / np.sqrt(v + LN_EPS) * sg.astype(np.float64) + sbe.astype(
        np.float64)
    h = np.maximum(h, 0.0)
    nl = h @ sW2.astype(np.float64).reshape(-1, 1) + float(np.asarray(sb2).reshape(-1)[0])
    n = np.clip(np.round(nl[:, 0]).astype(np.int64), 0, MAX_N - 1)
    return int(n.max())


def kernel(**inputs):
    ins = {k: np.ascontiguousarray(np.asarray(v)) for k, v in inputs.items()}
    z = ins["z"].astype(np.float32, copy=False)

    j_cap = min(MAX_N, _host_n_max(z, ins["sW1"], ins["sb1"], ins["sg"],
                                   ins["sbe"], ins["sW2"], ins["sb2"]) + 1)
    if j_cap not in _CACHE:
        _CACHE[j_cap] = _build(j_cap)
    nc = _CACHE[j_cap]

    weights = {
        "kW1": ins["kW1"], "kb1": ins["kb1"], "kg": ins["kg"],
        "kbe": ins["kbe"], "kW2": ins["kW2"], "kb2": ins["kb2"],
        "dW1": ins["dW1"], "db1": ins["db1"], "dW2": ins["dW2"],
        "db2": ins["db2"], "sW1": ins["sW1"], "sb1": ins["sb1"],
        "sg": ins["sg"], "sbe": ins["sbe"],
        "sW2": ins["sW2"].reshape(SM), "sb2": ins["sb2"].reshape(1),
    }
    weights = {k: np.ascontiguousarray(v, dtype=np.float32)
               for k, v in weights.items()}
    in_maps = [dict(weights, z=np.ascontiguousarray(z[c * BC:(c + 1) * BC]))
               for c in range(N_CORES)]

    res = run_bass_kernel_spmd(nc, in_maps, core_ids=list(range(N_CORES)))

    x_full = np.zeros((B, MAX_N, DIM), dtype=np.float32)
    n_logits = np.empty((B, 1), dtype=np.float32)
    n = np.empty((B,), dtype=np.int32)
    for c in range(N_CORES):
        r = res.results[c]
        x_full[c * BC:(c + 1) * BC, :j_cap, :] = r["xout"].reshape(
            BC, j_cap, DIM)
        n_logits[c * BC:(c + 1) * BC] = r["nlout"]
        n[c * BC:(c + 1) * BC] = r["nout"].reshape(BC)
    return x_full, n_logits, n


# revision 47
# speedup vs baseline: 2.7544x; 2.7544x over previous
"""Trainium2 Bass kernel for nn_Decoder (ragged_sequence).

Reference math (B=8192, HID=256, MAX_N=64, DIM=128):
    h        = relu(LN(z @ sW1 + sb1) * sg + sbe)          # [B, 128]
    n_logits = h @ sW2 + sb2                               # [B, 1]
    n        = clip(round(n_logits), 0, 63).int32          # [B]
    keys     = tanh(LN(kW1 + kb1) * kg + kbe) @ kW2 + kb2  # [64, HID]  (pos=I)
    x[b,j]   = (tanh((z[b]*keys[j]) @ dW1 + db1) @ dW2 + db2) * (j < n[b])

Ragged-aware structure: n is ~0-2 for this distribution, so almost all
of the dense [B, 64, 128] output is zeros.  The host runs an fp64
replica of the size-MLP to select, per slot j, the candidate rows with
nl > j + 0.5 - 0.01 (a margin far larger than any fp32 disagreement),
and the device decoder computes ONLY those packed rows.  The device
still computes n_logits/n for every row; the host scatters the packed
decoder output into the dense result using the DEVICE-computed n as the
mask, so correctness never depends on the host prediction (only
completeness does, via the margin).

Device-side layout choices:
  * keys is a pure function of the weights (identity positional
    encodings), so keysT -- and the per-slot scaled layer-1 weights
    dW1_j = diag(keys_j) @ dW1 -- are constant-folded on the host.
  * z is shipped pre-transposed (zT [HID, BC] per core) for the
    size-MLP; the decoder gets host-gathered packed columns zTsel.
  * The size-MLP LN runs on DVE only: bn_stats/bn_aggr for the stats
    and a quake-seed + 3-Newton-step rsqrt for 1/sd, so ACT stays
    decoder-only (tanh/copy all live in one activation table set --
    exactly one table load for the whole kernel).
  * Layer 2 uses the tanh activations as the stationary operand
    (lhsT = Y.T chunk, rhs = dW2ext), landing X batch-on-partition;
    db2 rides as an extra contraction row against a ones-row.
  * Packed x is written as contiguous 64KB [128, 128] blocks.

Sharding: pure data parallel over batch across 8 cores; weights
replicated.  All cores run one NEFF, so per-segment packed sizes are
padded to the max across cores (rounded up to 128).
"""

from contextlib import ExitStack

import numpy as np

import concourse.bacc as bacc
import concourse.bass as bass
import concourse.tile as tile
from concourse import mybir
from concourse.alu_op_type import AluOpType
from concourse.bass_utils import run_bass_kernel_spmd

F32 = mybir.dt.float32
I32 = mybir.dt.int32
AF = mybir.ActivationFunctionType

B, HID, MAX_N, DIM = 8192, 256, 64, 128
KM, DM, SM = 160, 192, 128
N_CORES = 8
BC = B // N_CORES          # 1024 rows per core
N_BT = BC // 128           # 8 batch tiles of 128 rows
LN_EPS = 1e-5
MAGIC = 12582912.0         # 1.5 * 2^23: fp32 round-to-nearest-even trick
SEL_MARGIN = 0.01          # >> any host/device fp32 n_logits disagreement

_CACHE: dict[tuple, object] = {}


def _layout_a(seg_sizes):
    """Decoder constants: [128, WA]."""
    cols = {}
    o = 0

    def add(name, w):
        nonlocal o
        cols[name] = (o, w)
        o += w

    for j in range(len(seg_sizes)):
        add(f"dW1j{j}_k0", DM)   # diag(keysT[:,j]) @ dW1, hid rows 0:128
        add(f"dW1j{j}_k1", DM)   # hid rows 128:256
    add("dW2e_k0", DIM)    # dW2ext[0:128, :]   (dW2ext = [dW2; db2], 193 rows)
    add("dW2e_k1", DIM)    # dW2ext[128:193, :] in rows 0:65
    add("db1c0", 1)        # db1[0:128] as a column
    add("db1c1", 1)        # db1[128:192] in rows 0:64
    return cols, o


def _layout_b():
    """Size-MLP constants: [128, WB]."""
    cols = {}
    o = 0

    def add(name, w):
        nonlocal o
        cols[name] = (o, w)
        o += w

    add("sW1_k0", SM)
    add("sW1_k1", SM)
    add("sb1_b", SM)          # sb1 row broadcast to all partitions
    add("sg_b", SM)
    add("sbe_b", SM)
    add("sW2_b", SM)          # sW2 row broadcast to all partitions
    add("eps", 1)
    add("sb2c", 1)
    return cols, o


def _fold_keys(ins) -> np.ndarray:
    """Host fp32 replica of the keys net (input-independent: pos = I)."""
    t = ins["kW1"] + ins["kb1"]
    mu = t.mean(-1, keepdims=True, dtype=np.float32)
    v = ((t - mu) ** 2).mean(-1, keepdims=True, dtype=np.float32)
    t = (t - mu) / np.sqrt(v + np.float32(LN_EPS)) * ins["kg"] + ins["kbe"]
    keys = np.tanh(t) @ ins["kW2"] + ins["kb2"]          # [64, HID]
    return np.ascontiguousarray(keys.T.astype(np.float32))  # [HID, 64]


def _pack_consts(ins, seg_sizes):
    ca, WA = _layout_a(seg_sizes)
    cb, WB = _layout_b()
    wa = np.zeros((128, WA), dtype=np.float32)
    wb = np.zeros((128, WB), dtype=np.float32)

    def put(w, cols, name, arr, rows=128):
        o, width = cols[name]
        w[:rows, o:o + width] = arr

    keysT = _fold_keys(ins)
    dW2e = np.concatenate([ins["dW2"], ins["db2"].reshape(1, DIM)], axis=0)
    for j in range(len(seg_sizes)):
        dW1j = ins["dW1"] * keysT[:, j:j + 1]
        put(wa, ca, f"dW1j{j}_k0", dW1j[0:128])
        put(wa, ca, f"dW1j{j}_k1", dW1j[128:256])
    put(wa, ca, "dW2e_k0", dW2e[0:128])
    put(wa, ca, "dW2e_k1", dW2e[128:193], rows=65)
    put(wa, ca, "db1c0", ins["db1"][0:128].reshape(128, 1))
    put(wa, ca, "db1c1", ins["db1"][128:192].reshape(64, 1), rows=64)

    put(wb, cb, "sW1_k0", ins["sW1"][0:128])
    put(wb, cb, "sW1_k1", ins["sW1"][128:256])
    put(wb, cb, "sb1_b", np.tile(ins["sb1"].reshape(1, SM), (128, 1)))
    put(wb, cb, "sg_b", np.tile(ins["sg"].reshape(1, SM), (128, 1)))
    put(wb, cb, "sbe_b", np.tile(ins["sbe"].reshape(1, SM), (128, 1)))
    put(wb, cb, "sW2_b", np.tile(ins["sW2"].reshape(1, SM), (128, 1)))
    put(wb, cb, "eps", np.full((128, 1), LN_EPS, dtype=np.float32))
    put(wb, cb, "sb2c",
        np.full((128, 1), np.float32(np.asarray(ins["sb2"]).reshape(-1)[0])))
    return wa, wb


def _build(seg_sizes: tuple):
    """Per-core Bass program; seg_sizes[j] = packed rows for slot j."""
    ca, WA = _layout_a(seg_sizes)
    cb, WB = _layout_b()
    nsel = int(sum(seg_sizes))
    nc = bacc.Bacc("TRN2", target_bir_lowering=False, debug=False,
                   num_devices=N_CORES)

    zT_d = nc.dram_tensor("zT", [HID, BC], F32, kind="ExternalInput").ap()
    wa_d = nc.dram_tensor("wpa", [128, WA], F32, kind="ExternalInput").ap()
    wb_d = nc.dram_tensor("wpb", [128, WB], F32, kind="ExternalInput").ap()
    if nsel:
        zs_d = nc.dram_tensor("zTsel", [HID, nsel], F32,
                              kind="ExternalInput").ap()
        xs_d = nc.dram_tensor("xsel", [nsel, DIM], F32,
                              kind="ExternalOutput").ap()
    # [p, bt] layout: batch row = bt*128 + p; host transposes back
    nl_d = nc.dram_tensor("nlout", [128, N_BT], F32, kind="ExternalOutput").ap()
    n_d = nc.dram_tensor("nout", [128, N_BT], I32, kind="ExternalOutput").ap()

    with tile.TileContext(nc) as tc, ExitStack() as es:
        singles = es.enter_context(tc.tile_pool(name="singles", bufs=1))
        work = es.enter_context(tc.tile_pool(name="work", bufs=3))
        yw = es.enter_context(tc.tile_pool(name="yw", bufs=4))
        xw = es.enter_context(tc.tile_pool(name="xw", bufs=8))
        ps_h = es.enter_context(tc.tile_pool(name="ps_h", bufs=2, space="PSUM"))
        ps_y0 = es.enter_context(tc.tile_pool(name="ps_y0", bufs=2, space="PSUM"))
        ps_y1 = es.enter_context(tc.tile_pool(name="ps_y1", bufs=2, space="PSUM"))
        ps_x = es.enter_context(tc.tile_pool(name="ps_x", bufs=2, space="PSUM"))

        # interleaved so both the size chain (wb+zT) and the decoder
        # (wa+zsel) start as early as possible
        wb = singles.tile([128, WB], F32, tag="wb")
        nc.sync.dma_start(out=wb, in_=wb_d)
        zT0 = singles.tile([128, BC], F32, tag="zT0")
        zT1 = singles.tile([128, BC], F32, tag="zT1")
        qsl = slice(0, BC // 4)
        nc.sync.dma_start(out=zT0[:, qsl], in_=zT_d[0:128, qsl])
        nc.sync.dma_start(out=zT1[:, qsl], in_=zT_d[128:256, qsl])
        wa = singles.tile([128, WA], F32, tag="wa")
        nc.sync.dma_start(out=wa, in_=wa_d)
        if nsel:
            zsel = singles.tile([128, 2, nsel], F32, tag="zsel")
            nc.sync.dma_start(out=zsel[:, 0, :], in_=zs_d[0:128, :])
            nc.sync.dma_start(out=zsel[:, 1, :], in_=zs_d[128:256, :])
        for q in range(1, 4):
            qsl = slice(q * (BC // 4), (q + 1) * (BC // 4))
            nc.sync.dma_start(out=zT0[:, qsl], in_=zT_d[0:128, qsl])
            nc.sync.dma_start(out=zT1[:, qsl], in_=zT_d[128:256, qsl])

        def A(name, rows=128):
            o, w = ca[name]
            return wa[0:rows, o:o + w]

        def Bc(name, rows=128):
            o, w = cb[name]
            return wb[0:rows, o:o + w]

        # ---------------- size MLP (dense; outputs nl and n) ----------------
        h_all = singles.tile([128, N_BT, SM], F32, tag="h_all")
        h2_all = singles.tile([128, N_BT, SM], F32, tag="h2_all")
        smv_all = singles.tile([128, N_BT, 2], F32, tag="smv_all")
        for bt in range(N_BT):
            sl = slice(bt * 128, (bt + 1) * 128)
            ph = ps_h.tile([128, SM], F32, tag="ph")
            nc.tensor.matmul(ph, lhsT=zT0[:, sl], rhs=Bc("sW1_k0"),
                             start=True, stop=False)
            nc.tensor.matmul(ph, lhsT=zT1[:, sl], rhs=Bc("sW1_k1"),
                             start=False, stop=True)
            nc.vector.tensor_tensor(out=h_all[:, bt, :], in0=ph,
                                    in1=Bc("sb1_b"), op=AluOpType.add)
            sstats = work.tile([128, 6], F32, tag="sstats")
            nc.vector.bn_stats(out=sstats, in_=h_all[:, bt, :])
            nc.vector.bn_aggr(out=smv_all[:, bt, :], in_=sstats)
        # rstd = rsqrt(var + eps) entirely on DVE (quake seed + 3 Newton
        # steps, exact to ~fp32 ulp here) -- keeps ACT decoder-only so its
        # in-order queue never stalls the decoder and the tanh/copy/relu
        # activation table set is loaded exactly once.  Two halves so the
        # first pipelines with the second half's arriving stats.
        H = N_BT // 2
        veps = work.tile([128, N_BT], F32, tag="veps")
        yi = work.tile([128, N_BT], I32, tag="yi")
        t1 = work.tile([128, N_BT], F32, tag="t1")
        rstd_all = yi.bitcast(F32)
        for hh in range(2):
            hs = slice(hh * H, (hh + 1) * H)
            nc.vector.tensor_scalar(out=veps[:, hs], in0=smv_all[:, hs, 1],
                                    scalar1=LN_EPS, scalar2=None,
                                    op0=AluOpType.add)
            # magic - (x>>1)  ==  ((x>>1) ^ -1) + (magic+1)
            nc.vector.tensor_scalar(out=yi[:, hs],
                                    in0=veps[:, hs].bitcast(I32), scalar1=1,
                                    scalar2=-1,
                                    op0=AluOpType.arith_shift_right,
                                    op1=AluOpType.bitwise_xor)
            nc.vector.tensor_scalar(out=yi[:, hs], in0=yi[:, hs],
                                    scalar1=0x5F3759E0, scalar2=None,
                                    op0=AluOpType.add)
            for _ in range(3):
                nc.vector.tensor_tensor(out=t1[:, hs], in0=rstd_all[:, hs],
                                        in1=rstd_all[:, hs],
                                        op=AluOpType.mult)
                nc.vector.tensor_tensor(out=t1[:, hs], in0=t1[:, hs],
                                        in1=veps[:, hs], op=AluOpType.mult)
                nc.vector.tensor_scalar(out=t1[:, hs], in0=t1[:, hs],
                                        scalar1=-0.5, scalar2=1.5,
                                        op0=AluOpType.mult,
                                        op1=AluOpType.add)
                nc.vector.tensor_tensor(out=rstd_all[:, hs],
                                        in0=rstd_all[:, hs], in1=t1[:, hs],
                                        op=AluOpType.mult)
        nl_all = work.tile([128, N_BT], F32, tag="nl_all")
        for bt in range(N_BT):
            nc.vector.scalar_tensor_tensor(out=h2_all[:, bt, :],
                                           in0=h_all[:, bt, :],
                                           scalar=smv_all[:, bt, 0:1],
                                           in1=Bc("sg_b"),
                                           op0=AluOpType.subtract,
                                           op1=AluOpType.mult)
            nc.vector.scalar_tensor_tensor(out=h_all[:, bt, :],
                                           in0=h2_all[:, bt, :],
                                           scalar=rstd_all[:, bt:bt + 1],
                                           in1=Bc("sbe_b"),
                                           op0=AluOpType.mult,
                                           op1=AluOpType.add)
            nc.vector.tensor_scalar(out=h2_all[:, bt, :],
                                    in0=h_all[:, bt, :], scalar1=0.0,
                                    scalar2=None, op0=AluOpType.max)
            nc.vector.tensor_tensor(out=h_all[:, bt, :],
                                    in0=h2_all[:, bt, :], in1=Bc("sW2_b"),
                                    op=AluOpType.mult)
            nc.vector.tensor_reduce(out=nl_all[:, bt:bt + 1],
                                    in_=h_all[:, bt, :],
                                    axis=mybir.AxisListType.X,
                                    op=AluOpType.add)
        nl2_all = work.tile([128, N_BT], F32, tag="nl2_all")
        nc.vector.tensor_scalar(out=nl2_all, in0=nl_all, scalar1=Bc("sb2c"),
                                scalar2=None, op0=AluOpType.add)
        nc.sync.dma_start(out=nl_d, in_=nl2_all)
        # n = clip(round_half_even(nl), 0, 63)
        nfa = work.tile([128, N_BT], F32, tag="nfa")
        nc.vector.tensor_scalar(out=nfa, in0=nl2_all, scalar1=MAGIC,
                                scalar2=None, op0=AluOpType.add)
        nfc = work.tile([128, N_BT], F32, tag="nfc")
        nc.vector.tensor_scalar(out=nfc, in0=nfa, scalar1=-MAGIC,
                                scalar2=float(MAX_N - 1), op0=AluOpType.add,
                                op1=AluOpType.min)
        nc.vector.tensor_scalar(out=nfc, in0=nfc, scalar1=0.0, scalar2=None,
                                op0=AluOpType.max)
        ni = work.tile([128, N_BT], I32, tag="ni")
        nc.vector.tensor_copy(out=ni, in_=nfc)
        nc.sync.dma_start(out=n_d, in_=ni)

        # ---------------- decoder over packed candidate rows ----------------
        off = 0
        for j, S in enumerate(seg_sizes):
            dW1j_k0 = A(f"dW1j{j}_k0")
            dW1j_k1 = A(f"dW1j{j}_k1")
            for c0 in range(0, S, 512):
                W = min(512, S - c0)
                ssl = slice(off + c0, off + c0 + W)
                py0 = ps_y0.tile([128, W], F32, tag="py0")
                nc.tensor.matmul(py0, lhsT=dW1j_k0[:, 0:128],
                                 rhs=zsel[:, 0, ssl], start=True, stop=False)
                nc.tensor.matmul(py0, lhsT=dW1j_k1[:, 0:128],
                                 rhs=zsel[:, 1, ssl], start=False, stop=True)
                py1 = ps_y1.tile([64, W], F32, tag="py1")
                nc.tensor.matmul(py1, lhsT=dW1j_k0[:, 128:192],
                                 rhs=zsel[:, 0, ssl], start=True, stop=False)
                nc.tensor.matmul(py1, lhsT=dW1j_k1[:, 128:192],
                                 rhs=zsel[:, 1, ssl], start=False, stop=True)
                y0 = yw.tile([128, 512], F32, tag="y0")
                y1e = yw.tile([65, 512], F32, tag="y1e")
                nc.scalar.activation(out=y0[:, 0:W], in_=py0, func=AF.Tanh,
                                     bias=A("db1c0"))
                nc.scalar.activation(out=y1e[0:64, 0:W], in_=py1,
                                     func=AF.Tanh, bias=A("db1c1", 64))
                nc.gpsimd.memset(y1e[64:65, 0:W], 1.0)
                for cc in range(0, W, 128):
                    px = ps_x.tile([128, DIM], F32, tag="px")
                    nc.tensor.matmul(px, lhsT=y0[:, cc:cc + 128],
                                     rhs=A("dW2e_k0"), start=True, stop=False)
                    nc.tensor.matmul(px, lhsT=y1e[:, cc:cc + 128],
                                     rhs=A("dW2e_k1", 65),
                                     start=False, stop=True)
                    xblk = xw.tile([128, DIM], F32, tag="xblk")
                    nc.scalar.activation(out=xblk, in_=px, func=AF.Copy)
                    row0 = off + c0 + cc
                    nc.sync.dma_start(out=xs_d[row0:row0 + 128, :], in_=xblk)
            off += S

    nc.compile()
    return nc


def _host_nl(z, sW1, sb1, sg, sbe, sW2, sb2):
    """fp64 replica of the size-MLP; plans the packed candidate sets."""
    h = z.astype(np.float64) @ sW1.astype(np.float64) + sb1.astype(np.float64)
    mu = h.mean(-1, keepdims=True)
    v = ((h - mu) ** 2).mean(-1, keepdims=True)
    h = (h - mu) / np.sqrt(v + LN_EPS) * sg.astype(np.float64) + sbe.astype(
        np.float64)
    h = np.maximum(h, 0.0)
    nl = h @ sW2.astype(np.float64).reshape(-1, 1) + float(
        np.asarray(sb2).reshape(-1)[0])
    return nl[:, 0]


def kernel(**inputs):
    ins = {k: np.ascontiguousarray(np.asarray(v), dtype=np.float32)
           for k, v in inputs.items()}
    z = ins["z"]

    nl_host = _host_nl(z, ins["sW1"], ins["sb1"], ins["sg"], ins["sbe"],
                       ins["sW2"], ins["sb2"])
    # candidate rows per slot, per core: a superset of the truly valid
    # rows as long as |nl_host - nl_device| < SEL_MARGIN
    sels = []   # sels[j][c] = np.array of core-local row indices
    seg_sizes = []
    for j in range(MAX_N):
        cand = nl_host > (j + 0.5 - SEL_MARGIN)
        if not cand.any():
            break
        per_core = [np.nonzero(cand[c * BC:(c + 1) * BC])[0]
                    for c in range(N_CORES)]
        sels.append(per_core)
        mx = max(len(p) for p in per_core)
        seg_sizes.append(max(128, -(-mx // 128) * 128))
    seg_sizes = tuple(seg_sizes)
    nsel = int(sum(seg_sizes))

    if seg_sizes not in _CACHE:
        _CACHE[seg_sizes] = _build(seg_sizes)
    nc = _CACHE[seg_sizes]

    wa, wb = _pack_consts(ins, seg_sizes)
    in_maps = []
    for c in range(N_CORES):
        m = {"zT": np.ascontiguousarray(z[c * BC:(c + 1) * BC].T),
             "wpa": wa, "wpb": wb}
        if nsel:
            zs = np.zeros((HID, nsel), dtype=np.float32)
            off = 0
            for j, S in enumerate(seg_sizes):
                rows = sels[j][c]
                zs[:, off:off + len(rows)] = z[c * BC + rows].T
                off += S
            m["zTsel"] = zs
        in_maps.append(m)

    res = run_bass_kernel_spmd(nc, in_maps, core_ids=list(range(N_CORES)))

    x_full = np.zeros((B, MAX_N, DIM), dtype=np.float32)
    n_logits = np.empty((B, 1), dtype=np.float32)
    n = np.empty((B,), dtype=np.int32)
    for c in range(N_CORES):
        r = res.results[c]
        n_logits[c * BC:(c + 1) * BC, 0] = r["nlout"].T.reshape(BC)
        n_c = r["nout"].T.reshape(BC)
        n[c * BC:(c + 1) * BC] = n_c
        if nsel:
            xs = r["xsel"]
            off = 0
            for j, S in enumerate(seg_sizes):
                rows = sels[j][c]            # core-local indices
                valid = n_c[rows] > j        # DEVICE n decides validity
                x_full[c * BC + rows[valid], j, :] = \
                    xs[off:off + len(rows)][valid]
                off += S
    return x_full, n_logits, n
